# revision 2
# baseline (speedup 1.0000x reference)
"""AUGRU cell kernel for trn2, 8-core data-parallel, fp8-DoubleRow compute.

Layout: transposed ("feature-major") - features on partitions, batch rows on
the free dim; per-feature gate biases become per-partition ACT biases.

All six gemm-sides (x@Wu, h@Uu, x@Wr, h@Ur, h@Uh, x@Wh) run as fp8-e4m3
DoubleRow matmuls (K=256 per instruction at 0.5 cycles/row = 4x bf16 rate)
with full Dekker compensation per side, using a global x16 weight prescale:

    S*(v @ W) ~= v8 @ Q8(S*W) + v8 @ Q8(S*W - Q8(S*W)) + vr8 @ Q8(W)

where v8 = Q8(v) and vr8 = Q8(S*(v - v8)) carries the S factor. The S scale
keeps the weight residual inside e4m3's normal range and is undone for free
by the activations' input scale (sigmoid/tanh compute f(psum/S + bias)).
Measured end-to-end error vs the fp32 reference: ~1.5e-3 (vs 2e-2 budget).

Per (1024-row chunk, m-half) iteration, PE order rx,rh,e,ux,uh,g (36 DR
matmuls into 4 psum tiles of 2 banks each = all 8 banks):
  u  = sigmoid(ps_u/S + bu)                  [ScalarE, 2-bank psum read]
  r  = sigmoid(ps_r/S + br)                  [ScalarE]
  s  = a_bc * u                              [DVE stt, fp16 4x]
  t  = r * ps_e                              [DVE stt]
  hp = t + ps_g                              [DVE stt, fp32 out]
  hh = tanh(hp/S + bh)                       [ScalarE]
  d  = hh - h16 ; p = s * d ; out = h16 + p  [DVE stt, fp16 4x]
All pointwise ops are scalar_tensor_tensor (supports the 4x_2p DVE perf
mode; plain tensor_tensor only supports 2x_1p). Intermediates are fp16
(~6x lower rounding error than bf16); output is stored fp16 and upcast on
the host. a is broadcast across partitions by a stride-0 DMA.
"""
import numpy as np
from contextlib import ExitStack

import bass_rust
import ml_dtypes
import concourse.bass as bass
import concourse.mybir as mybir
import concourse.tile as tile
from concourse.bass_utils import run_bass_kernel_spmd

F32 = mybir.dt.float32
F16 = mybir.dt.float16
F8 = mybir.dt.float8e4
E4 = ml_dtypes.float8_e4m3
DR = mybir.MatmulPerfMode.DoubleRow
SIG = mybir.ActivationFunctionType.Sigmoid
TANH = mybir.ActivationFunctionType.Tanh

B, D = 65536, 256
NCORES = 8
BL = B // NCORES          # rows per core
P = 128
CN = 1024                 # rows per chunk
NCHUNK = BL // CN         # 8
S = 16.0                  # global weight prescale

# gemm-side order on the PE: r-gate first so its sigmoid (feeding the long
# t->hp->tanh chain) starts as early as possible.
# (side index, moving per term: 0,1 -> v8, 2 -> vr8)
SIDES = ["rx", "rh", "e", "ux", "uh", "g"]
X_SIDES = {"rx", "ux", "g"}
PS_OF = {"rx": "r", "rh": "r", "ux": "u", "uh": "u", "e": "e", "g": "g"}
NT = 3                    # terms per side


def split_multi_waits(nc):
    """Walrus codegen allows at most one semaphore wait per instruction.
    Split extras onto preceding same-engine NoOps."""
    for fn in nc.m.functions:
        for bb in fn.blocks:
            out = []
            for inst in bb.instructions:
                si = inst.sync_info
                if si is not None and len(si.on_wait) > 1:
                    waits = list(si.on_wait)
                    for j, w in enumerate(waits[:-1]):
                        nop = bass_rust.InstNoOp(name=f"{inst.name}-sw{j}")
                        nop.engine = inst.engine
                        nop.sync_info = mybir.SyncInfo(on_wait=[w], on_update=[])
                        out.append(nop)
                    inst.sync_info = mybir.SyncInfo(
                        on_wait=[waits[-1]], on_update=list(si.on_update))
                out.append(inst)
            bb.instructions = out


def build():
    nc = bass.Bass()
    c_d = nc.declare_dram_parameter("consts8", [P, 6 * NT * 2 * 256], F8, isOutput=False)
    b_d = nc.declare_dram_parameter("bias", [P, 6], F32, isOutput=False)
    x8_d = nc.declare_dram_parameter("x8T", [D, BL], F8, isOutput=False)
    xr_d = nc.declare_dram_parameter("xrT", [D, BL], F8, isOutput=False)
    h8_d = nc.declare_dram_parameter("h8T", [D, BL], F8, isOutput=False)
    hr_d = nc.declare_dram_parameter("hrT", [D, BL], F8, isOutput=False)
    h16_d = nc.declare_dram_parameter("h16T", [D, BL], F16, isOutput=False)
    a_d = nc.declare_dram_parameter("aT", [1, BL], F16, isOutput=False)
    o_d = nc.declare_dram_parameter("outT", [D, BL], F16, isOutput=True)

    c_ap = c_d.ap().rearrange("p (s t k m) -> p s t k m", s=6, t=NT, k=2)
    x8_ap = x8_d.ap().rearrange("(c p) n -> p c n", p=P)
    xr_ap = xr_d.ap().rearrange("(c p) n -> p c n", p=P)
    h8_ap = h8_d.ap().rearrange("(c p) n -> p c n", p=P)
    hr_ap = hr_d.ap().rearrange("(c p) n -> p c n", p=P)
    h16_ap = h16_d.ap().rearrange("(c p) n -> p c n", p=P)
    o_ap = o_d.ap().rearrange("(c p) n -> p c n", p=P)

    with tile.TileContext(nc) as tc, ExitStack() as ctx:
        const = ctx.enter_context(tc.tile_pool(name="const", bufs=1))
        io = ctx.enter_context(tc.tile_pool(name="io", bufs=1))
        sm = ctx.enter_context(tc.tile_pool(name="sm", bufs=1))
        psum = ctx.enter_context(tc.tile_pool(name="psum", bufs=1, space="PSUM"))

        c_sb = const.tile([P, 6, NT, 2, 256], F8)
        b_sb = const.tile([P, 6], F32)
        # startup: r-gate weights, first chunk's moving data, rest of weights
        nc.sync.dma_start(out=c_sb[:, 0:2], in_=c_ap[:, 0:2])        # rx, rh
        nc.sync.dma_start(out=b_sb, in_=b_d.ap())

        def load_chunk(ci):
            n0 = ci * CN
            tiles = {}
            for nm, ap, dt_ in (("x8", x8_ap, F8), ("h8", h8_ap, F8),
                                ("xr", xr_ap, F8), ("hr", hr_ap, F8),
                                ("h16", h16_ap, F16)):
                t_ = io.tile([P, 2, CN], dt_, tag=nm, bufs=3, name=nm)
                nc.sync.dma_start(out=t_, in_=ap[:, :, n0:n0 + CN])
                tiles[nm] = t_
            a_bc = io.tile([P, CN], F16, tag="a_bc", bufs=3, name="a_bc")
            nc.sync.dma_start(out=a_bc,
                              in_=a_d.ap()[0:1, n0:n0 + CN].to_broadcast((P, CN)))
            tiles["a_bc"] = a_bc
            return tiles

        t0 = load_chunk(0)
        nc.sync.dma_start(out=c_sb[:, 2:6], in_=c_ap[:, 2:6])        # e, ux, uh, g

        for ci in range(NCHUNK):
            tl = t0 if ci == 0 else load_chunk(ci)
            mv_of = {"v8": {True: tl["x8"], False: tl["h8"]},
                     "vr": {True: tl["xr"], False: tl["hr"]}}
            for m in range(2):
                ms = slice(m * P, (m + 1) * P)
                ps = {k: psum.tile([P, 2, 512], F32, tag=f"ps_{k}", bufs=1,
                                   name=f"ps_{k}")
                      for k in ("r", "u", "e", "g")}
                # PE: 36 DoubleRow matmuls, grouped per psum bank
                for si, side in enumerate(SIDES):
                    isx = side in X_SIDES
                    pst = ps[PS_OF[side]]
                    two_sided = side[0] in ("r", "u")
                    first_side = side in ("rx", "ux") or not two_sided
                    last_side = side in ("rh", "uh") or not two_sided
                    for j in range(2):
                        js = slice(j * 512, (j + 1) * 512)
                        for t_ in range(NT):
                            mv = mv_of["vr" if t_ == 2 else "v8"][isx]
                            nc.tensor.matmul(
                                pst[:, j, :], c_sb[:, si, t_, :, ms],
                                mv[:, :, js],
                                start=(first_side and t_ == 0),
                                stop=(last_side and t_ == NT - 1),
                                perf_mode=DR)

                ps_flat = {k: v.rearrange("p a b -> p (a b)")
                           for k, v in ps.items()}
                r16 = sm.tile([P, CN], F16, tag="r16", bufs=2, name="r16")
                nc.scalar.activation(r16, ps_flat["r"], SIG,
                                     bias=b_sb[:, 2 + m:3 + m], scale=1.0 / S)
                u16 = sm.tile([P, CN], F16, tag="u16", bufs=2, name="u16")
                nc.scalar.activation(u16, ps_flat["u"], SIG,
                                     bias=b_sb[:, 0 + m:1 + m], scale=1.0 / S)
                t16 = sm.tile([P, CN], F16, tag="t16", bufs=2, name="t16")
                nc.vector.scalar_tensor_tensor(
                    out=t16, in0=r16, scalar=1.0, in1=ps_flat["e"],
                    op0=mybir.AluOpType.mult, op1=mybir.AluOpType.mult)
                s16 = sm.tile([P, CN], F16, tag="s16", bufs=2, name="s16")
                nc.vector.scalar_tensor_tensor(
                    out=s16, in0=tl["a_bc"], scalar=1.0, in1=u16,
                    op0=mybir.AluOpType.mult, op1=mybir.AluOpType.mult)
                hp = sm.tile([P, CN], F32, tag="hp", bufs=2, name="hp")
                nc.vector.scalar_tensor_tensor(
                    out=hp, in0=t16, scalar=1.0, in1=ps_flat["g"],
                    op0=mybir.AluOpType.mult, op1=mybir.AluOpType.add)
                hh = sm.tile([P, CN], F16, tag="hh", bufs=2, name="hh")
                nc.scalar.activation(hh, hp, TANH,
                                     bias=b_sb[:, 4 + m:5 + m], scale=1.0 / S)
                d16 = sm.tile([P, CN], F16, tag="d16", bufs=2, name="d16")
                nc.vector.scalar_tensor_tensor(
                    out=d16, in0=hh, scalar=1.0, in1=tl["h16"][:, m, :],
                    op0=mybir.AluOpType.mult, op1=mybir.AluOpType.subtract)
                p16 = sm.tile([P, CN], F16, tag="p16", bufs=2, name="p16")
                nc.vector.scalar_tensor_tensor(
                    out=p16, in0=s16, scalar=1.0, in1=d16,
                    op0=mybir.AluOpType.mult, op1=mybir.AluOpType.mult)
                o16 = sm.tile([P, CN], F16, tag=f"o16_{m}", bufs=2, name="o16")
                nc.vector.scalar_tensor_tensor(
                    out=o16, in0=tl["h16"][:, m, :], scalar=1.0, in1=p16,
                    op0=mybir.AluOpType.mult, op1=mybir.AluOpType.add)
                nc.sync.dma_start(out=o_ap[:, m, ci * CN:(ci + 1) * CN], in_=o16)

    split_multi_waits(nc)
    return nc


def q8(v):
    return np.asarray(v, np.float32).astype(E4)


def pack_consts(Wu, Uu, bu, Wr, Ur, br, Wh, Uh, bh):
    """consts8 [P, 6, 3, 2, 256] fp8 + bias [P, 6] fp32."""
    w_of = {"rx": Wr, "rh": Ur, "e": Uh, "ux": Wu, "uh": Uu, "g": Wh}
    out = np.zeros((P, 6, NT, 2, 256), E4)

    def ktile(arr):  # [256, 256] -> [128, 2, 256]
        return np.asarray(arr).reshape(2, P, 256).transpose(1, 0, 2)

    for si, side in enumerate(SIDES):
        W = np.asarray(w_of[side], np.float32)
        main = q8(S * W)
        wres = q8(S * W - main.astype(np.float32))
        acres = q8(W)
        out[:, si, 0] = ktile(main)
        out[:, si, 1] = ktile(wres)
        out[:, si, 2] = ktile(acres)

    bias = np.zeros((P, 6), np.float32)
    for gi, bv in enumerate((bu, br, bh)):
        bias[:, 2 * gi:2 * gi + 2] = np.asarray(bv, np.float32).reshape(2, P).T
    return np.ascontiguousarray(out.reshape(P, -1)), np.ascontiguousarray(bias)


_CACHE = {}


def _get_nc():
    if "nc" not in _CACHE:
        _CACHE["nc"] = build()
    return _CACHE["nc"]


def kernel(x, h_1, a, Wu, Uu, bu, Wr, Ur, br, Wh, Uh, bh):
    nc = _get_nc()
    consts8, bias = pack_consts(Wu, Uu, bu, Wr, Ur, br, Wh, Uh, bh)
    x = np.asarray(x, np.float32)
    h = np.asarray(h_1, np.float32)
    a = np.asarray(a, np.float32)

    x8 = x.astype(E4)
    xr = (S * (x - x8.astype(np.float32))).astype(E4)
    h8 = h.astype(E4)
    hr = (S * (h - h8.astype(np.float32))).astype(E4)
    h16 = h.astype(np.float16)

    in_maps = []
    for c in range(NCORES):
        sl = slice(c * BL, (c + 1) * BL)
        in_maps.append({
            "consts8": consts8,
            "bias": bias,
            "x8T": np.ascontiguousarray(x8[sl].T),
            "xrT": np.ascontiguousarray(xr[sl].T),
            "h8T": np.ascontiguousarray(h8[sl].T),
            "hrT": np.ascontiguousarray(hr[sl].T),
            "h16T": np.ascontiguousarray(h16[sl].T),
            "aT": np.ascontiguousarray(a[sl].T).astype(np.float16),
        })
    res = run_bass_kernel_spmd(nc, in_maps, list(range(NCORES)))
    out = np.empty((B, D), np.float32)
    for c in range(NCORES):
        out[c * BL:(c + 1) * BL] = np.asarray(res.results[c]["outT"]).T.astype(np.float32)
    return out


# revision 10
# speedup vs baseline: 1.7309x; 1.7309x over previous
"""AUGRU cell kernel for trn2, 8-core data-parallel, fp8-DoubleRow compute.

Layout: transposed ("feature-major") - features on partitions, batch rows on
the free dim; per-feature gate biases become per-partition ACT biases.

All six gemm-sides (x@Wu, h@Uu, x@Wr, h@Ur, h@Uh, x@Wh) run as fp8-e4m3
DoubleRow matmuls (K=256 per instruction at 0.5 cycles/row = 4x bf16 rate)
with full Dekker compensation per side, using a global x16 weight prescale:

    S*(v @ W) ~= v8 @ Q8(S*W) + v8 @ Q8(S*W - Q8(S*W)) + vr8 @ Q8(W)

where v8 = Q8(v) and vr8 = Q8(S*(v - v8)) carries the S factor. The S scale
keeps the weight residual inside e4m3's normal range and is undone for free
by the activations' input scale (sigmoid/tanh compute f(psum/S + bias)).
Measured end-to-end error vs the fp32 reference: ~1.5e-3 (vs 2e-2 budget).

Per (1024-row chunk, m-half) iteration, PE order rx,rh,e,ux,uh,g (36 DR
matmuls into 4 psum tiles of 2 banks each = all 8 banks):
  u  = sigmoid(ps_u/S + bu)                  [ScalarE, 2-bank psum read]
  r  = sigmoid(ps_r/S + br)                  [ScalarE]
  s  = a_bc * u                              [DVE stt, fp16 4x]
  t  = r * ps_e                              [DVE stt]
  hp = t + ps_g                              [DVE stt, fp32 out]
  hh = tanh(hp/S + bh)                       [ScalarE]
  d  = hh - h16 ; p = s * d ; out = h16 + p  [DVE stt, fp16 4x]
All pointwise ops are scalar_tensor_tensor (supports the 4x_2p DVE perf
mode; plain tensor_tensor only supports 2x_1p). Intermediates are fp16
(~6x lower rounding error than bf16); output is stored fp16 and upcast on
the host. a is broadcast across partitions by a stride-0 DMA.
"""
import numpy as np
from contextlib import ExitStack

import bass_rust
import ml_dtypes
import concourse.bass as bass
import concourse.mybir as mybir
import concourse.tile as tile
from concourse.bass_utils import run_bass_kernel_spmd

F32 = mybir.dt.float32
F16 = mybir.dt.float16
F8 = mybir.dt.float8e4
E4 = ml_dtypes.float8_e4m3
DR = mybir.MatmulPerfMode.DoubleRow
SIG = mybir.ActivationFunctionType.Sigmoid
TANH = mybir.ActivationFunctionType.Tanh

B, D = 65536, 256
NCORES = 8
BL = B // NCORES          # rows per core
P = 128
CN = 1024                 # rows per chunk
NCHUNK = BL // CN         # 8
S = 16.0                  # global weight prescale

# gemm-side order on the PE: r-gate first so its sigmoid (feeding the long
# t->hp->tanh chain) starts as early as possible.
# (side index, moving per term: 0,1 -> v8, 2 -> vr8)
SIDES = ["rx", "rh", "e", "ux", "uh", "g"]
X_SIDES = {"rx", "ux", "g"}
PS_OF = {"rx": "r", "rh": "r", "ux": "u", "uh": "u", "e": "e", "g": "g"}
NT = 3                    # terms per side
# per-side compensation terms: 0=main Q8(S*W), 1=weight-residual, 2=act-residual.
# rx drops its weight residual (measured composite rel err 9.3e-3 vs 2e-2 budget).
TERMS = {"rx": [0, 2], "rh": [0, 1, 2], "e": [0, 1, 2],
         "ux": [0, 1, 2], "uh": [0, 1, 2], "g": [0, 1, 2]}


def split_multi_waits(nc):
    """Walrus codegen allows at most one semaphore wait per instruction.
    Split extras onto preceding same-engine NoOps."""
    for fn in nc.m.functions:
        for bb in fn.blocks:
            out = []
            for inst in bb.instructions:
                si = inst.sync_info
                if si is not None and len(si.on_wait) > 1:
                    waits = list(si.on_wait)
                    for j, w in enumerate(waits[:-1]):
                        nop = bass_rust.InstNoOp(name=f"{inst.name}-sw{j}")
                        nop.engine = inst.engine
                        nop.sync_info = mybir.SyncInfo(on_wait=[w], on_update=[])
                        out.append(nop)
                    inst.sync_info = mybir.SyncInfo(
                        on_wait=[waits[-1]], on_update=list(si.on_update))
                out.append(inst)
            bb.instructions = out


def build():
    nc = bass.Bass()
    c_d = nc.declare_dram_parameter("consts8", [P, 6 * NT * 2 * 256], F8, isOutput=False)
    b_d = nc.declare_dram_parameter("bias", [P, 6], F32, isOutput=False)
    x8_d = nc.declare_dram_parameter("x8T", [D, BL], F8, isOutput=False)
    xr_d = nc.declare_dram_parameter("xrT", [D, BL], F8, isOutput=False)
    h8_d = nc.declare_dram_parameter("h8T", [D, BL], F8, isOutput=False)
    hr_d = nc.declare_dram_parameter("hrT", [D, BL], F8, isOutput=False)
    h16_d = nc.declare_dram_parameter("h16T", [D, BL], F16, isOutput=False)
    a_d = nc.declare_dram_parameter("aT", [1, BL], F16, isOutput=False)
    o_d = nc.declare_dram_parameter("outT", [D, BL], F16, isOutput=True)

    c_ap = c_d.ap().rearrange("p (h s t k m) -> p h s t k m", h=2, s=6, t=NT, k=2)
    aps = {"x8": x8_d.ap().rearrange("(c p) n -> p c n", p=P),
           "xr": xr_d.ap().rearrange("(c p) n -> p c n", p=P),
           "h8": h8_d.ap().rearrange("(c p) n -> p c n", p=P),
           "hr": hr_d.ap().rearrange("(c p) n -> p c n", p=P),
           "h16": h16_d.ap().rearrange("(c p) n -> p c n", p=P)}
    o_ap = o_d.ap().rearrange("(c p) n -> p c n", p=P)

    # chunk schedule: small first/last chunks for fast pipeline fill/drain
    chunks = []
    n0 = 0
    for cn in [512] + [1024] * 7 + [512]:
        chunks.append((n0, cn))
        n0 += cn
    assert n0 == BL

    dt_of = {"x8": F8, "xr": F8, "h8": F8, "hr": F8, "h16": F16}

    with tile.TileContext(nc) as tc, ExitStack() as ctx:
        const = ctx.enter_context(tc.tile_pool(name="const", bufs=1))
        io = ctx.enter_context(tc.tile_pool(name="io", bufs=1))
        sm = ctx.enter_context(tc.tile_pool(name="sm", bufs=1))
        psum = ctx.enter_context(tc.tile_pool(name="psum", bufs=1, space="PSUM"))

        c_sb = const.tile([P, 2, 6, NT, 2, 128], F8)
        b_sb = const.tile([P, 6], F32)

        def load_one(nm, ci, n0, cn):
            t_ = io.tile([P, 2, CN], dt_of[nm], tag=nm, bufs=4, name=nm)
            _lbl(nc.sync.dma_start(out=t_[:, :, 0:cn], in_=aps[nm][:, :, n0:n0 + cn]),
                 f"dma.{nm}.c{ci}")
            return t_

        def load_abc(ci, n0, cn):
            a_bc = io.tile([P, CN], F16, tag="a_bc", bufs=4, name="a_bc")
            _lbl(nc.sync.dma_start(
                out=a_bc[:, 0:cn],
                in_=a_d.ap()[0:1, n0:n0 + cn].to_broadcast((P, cn))), f"dma.a.c{ci}")
            return a_bc

        # staged startup: weights interleaved with first-chunk data in PE need order
        n0_0, cn_0 = chunks[0]
        nc.sync.dma_start(out=c_sb[:, 0:1, 0:3], in_=c_ap[:, 0:1, 0:3])  # m0 rx,rh,e w
        t0 = {"x8": load_one("x8", 0, n0_0, cn_0)}
        t0["h8"] = load_one("h8", 0, n0_0, cn_0)
        t0["xr"] = load_one("xr", 0, n0_0, cn_0)
        t0["hr"] = load_one("hr", 0, n0_0, cn_0)
        nc.sync.dma_start(out=c_sb[:, 0:1, 3:6], in_=c_ap[:, 0:1, 3:6])  # m0 ux,uh,g w
        nc.sync.dma_start(out=b_sb, in_=b_d.ap())
        nc.sync.dma_start(out=c_sb[:, 1:2], in_=c_ap[:, 1:2])        # m1 weights
        t0["h16"] = load_one("h16", 0, n0_0, cn_0)
        t0["a_bc"] = load_abc(0, n0_0, cn_0)

        for ci, (n0, cn) in enumerate(chunks):
            nj = max(1, cn // 512)
            if ci == 0:
                tl = t0
            else:
                tl = {nm: load_one(nm, ci, n0, cn) for nm in
                      ("x8", "h8", "xr", "hr", "h16")}
                tl["a_bc"] = load_abc(ci, n0, cn)
            mv_of = {0: {True: tl["x8"], False: tl["h8"]},
                     2: {True: tl["xr"], False: tl["hr"]}}
            mv_of[1] = mv_of[0]
            s16 = sm.tile([P, 2, CN], F16, tag="s16", bufs=2, name="s16")
            hh = sm.tile([P, 2, CN], F16, tag="hh", bufs=2, name="hh")
            for m in range(2):
                ps = {k: psum.tile([P, 2, 512], F32, tag=f"ps_{k}", bufs=1,
                                   name=f"ps_{k}")
                      for k in ("r", "u", "e", "g")}
                for si, side in enumerate(SIDES):
                    isx = side in X_SIDES
                    pst = ps[PS_OF[side]]
                    terms = TERMS[side]
                    two_sided = side[0] in ("r", "u")
                    first_side = side in ("rx", "ux") or not two_sided
                    last_side = side in ("rh", "uh") or not two_sided
                    for j in range(nj):
                        jw = min(512, cn - j * 512)
                        js = slice(j * 512, j * 512 + jw)
                        for t_ in terms:
                            mv = mv_of[t_][isx]
                            _lbl(nc.tensor.matmul(
                                pst[:, j, 0:jw], c_sb[:, m, si, t_, :, :],
                                mv[:, :, js],
                                start=(first_side and t_ == terms[0]),
                                stop=(last_side and t_ == terms[-1]),
                                perf_mode=DR), f"mm.{side}.{t_}.j{j}.c{ci}m{m}")

                ps_flat = {k: v.rearrange("p a b -> p (a b)")[:, 0:cn]
                           for k, v in ps.items()}
                r16 = sm.tile([P, CN], F16, tag="r16", bufs=2, name="r16")
                _lbl(nc.scalar.activation(r16[:, 0:cn], ps_flat["r"], SIG,
                                          bias=b_sb[:, 2 + m:3 + m], scale=1.0 / S),
                     f"act.r.c{ci}m{m}")
                u16 = sm.tile([P, CN], F16, tag="u16", bufs=2, name="u16")
                _lbl(nc.scalar.activation(u16[:, 0:cn], ps_flat["u"], SIG,
                                          bias=b_sb[:, 0 + m:1 + m], scale=1.0 / S),
                     f"act.u.c{ci}m{m}")
                t16 = sm.tile([P, CN], F16, tag="t16", bufs=2, name="t16")
                _lbl(nc.vector.tensor_mul(out=t16[:, 0:cn], in0=r16[:, 0:cn],
                                          in1=ps_flat["e"]), f"dve.t.c{ci}m{m}")
                _lbl(nc.gpsimd.tensor_mul(out=s16[:, m, 0:cn], in0=tl["a_bc"][:, 0:cn],
                                          in1=u16[:, 0:cn]), f"pool.s.c{ci}m{m}")
                hp = sm.tile([P, CN], F32, tag="hp", bufs=2, name="hp")
                _lbl(nc.vector.tensor_add(out=hp[:, 0:cn], in0=t16[:, 0:cn],
                                          in1=ps_flat["g"]), f"dve.hp.c{ci}m{m}")
                _lbl(nc.scalar.activation(hh[:, m, 0:cn], hp[:, 0:cn], TANH,
                                          bias=b_sb[:, 4 + m:5 + m], scale=1.0 / S),
                     f"act.tanh.c{ci}m{m}")

            # blend per m-half right after each tanh (keeps the drain tail short)
            last = ci == len(chunks) - 1
            for m in range(2):
                d16 = sm.tile([P, CN], F16, tag="d16", bufs=2, name="d16")
                _lbl(nc.vector.tensor_sub(out=d16[:, 0:cn], in0=hh[:, m, 0:cn],
                                          in1=tl["h16"][:, m, 0:cn]), f"dve.d.c{ci}m{m}")
                p16 = sm.tile([P, CN], F16, tag="p16", bufs=2, name="p16")
                _lbl(nc.vector.tensor_mul(out=p16[:, 0:cn], in0=s16[:, m, 0:cn],
                                          in1=d16[:, 0:cn]), f"dve.p.c{ci}m{m}")
                if not last:
                    # out = h16 + p via fp16 accumulating DMA onto prefilled outT
                    _lbl(nc.gpsimd.dma_start(
                        out=o_ap[:, m, n0:n0 + cn],
                        in_=p16[:, 0:cn],
                        accum_op=mybir.AluOpType.add), f"dma.out.c{ci}m{m}")
                else:
                    # drain tail: DVE out-add + plain store (no SWDGE chain)
                    o16 = sm.tile([P, CN], F16, tag="o16l", bufs=2, name="o16l")
                    _lbl(nc.vector.tensor_add(out=o16[:, 0:cn],
                                              in0=tl["h16"][:, m, 0:cn],
                                              in1=p16[:, 0:cn]), f"dve.o.c{ci}m{m}")
                    _lbl(nc.sync.dma_start(out=o_ap[:, m, n0:n0 + cn],
                                           in_=o16[:, 0:cn]), f"dma.out.c{ci}m{m}")

    split_multi_waits(nc)
    return nc


def q8(v):
    return np.asarray(v, np.float32).astype(E4)


def pack_consts(Wu, Uu, bu, Wr, Ur, br, Wh, Uh, bh):
    """consts8 [P, 2, 6, 3, 2, 128] fp8 (m-half major) + bias [P, 6] fp32."""
    w_of = {"rx": Wr, "rh": Ur, "e": Uh, "ux": Wu, "uh": Uu, "g": Wh}
    out = np.zeros((P, 2, 6, NT, 2, 128), E4)

    def ktile(arr):  # [256, 256] -> [128, 2, 256]
        return np.asarray(arr).reshape(2, P, 256).transpose(1, 0, 2)

    for si, side in enumerate(SIDES):
        W = np.asarray(w_of[side], np.float32)
        main = q8(S * W)
        wres = q8(S * W - main.astype(np.float32))
        acres = q8(W)
        for t_, arr in enumerate((main, wres, acres)):
            kt = ktile(arr)           # [128, 2, 256]
            for mh in range(2):
                out[:, mh, si, t_] = kt[:, :, mh * 128:(mh + 1) * 128]

    bias = np.zeros((P, 6), np.float32)
    for gi, bv in enumerate((bu, br, bh)):
        bias[:, 2 * gi:2 * gi + 2] = np.asarray(bv, np.float32).reshape(2, P).T
    return np.ascontiguousarray(out.reshape(P, -1)), np.ascontiguousarray(bias)


_CACHE = {}
LABELS = {}


def _lbl(inst, label):
    try:
        LABELS[inst.name] = label
    except Exception:
        pass
    return inst


def _get_nc():
    if "nc" not in _CACHE:
        _CACHE["nc"] = build()
    return _CACHE["nc"]


def kernel(x, h_1, a, Wu, Uu, bu, Wr, Ur, br, Wh, Uh, bh):
    nc = _get_nc()
    consts8, bias = pack_consts(Wu, Uu, bu, Wr, Ur, br, Wh, Uh, bh)
    x = np.asarray(x, np.float32)
    h = np.asarray(h_1, np.float32)
    a = np.asarray(a, np.float32)

    x8 = x.astype(E4)
    xr = (S * (x - x8.astype(np.float32))).astype(E4)
    h8 = h.astype(E4)
    hr = (S * (h - h8.astype(np.float32))).astype(E4)
    h16 = h.astype(np.float16)

    in_maps = []
    for c in range(NCORES):
        sl = slice(c * BL, (c + 1) * BL)
        in_maps.append({
            "consts8": consts8,
            "bias": bias,
            "x8T": np.ascontiguousarray(x8[sl].T),
            "xrT": np.ascontiguousarray(xr[sl].T),
            "h8T": np.ascontiguousarray(h8[sl].T),
            "hrT": np.ascontiguousarray(hr[sl].T),
            "h16T": np.ascontiguousarray(h16[sl].T),
            "aT": np.ascontiguousarray(a[sl].T).astype(np.float16),
        })
    prefills = [{"outT": im["h16T"]} for im in in_maps]
    results = run_spmd_prefill(nc, in_maps, prefills, NCORES)
    out = np.empty((B, D), np.float32)
    for c in range(NCORES):
        out[c * BL:(c + 1) * BL] = np.asarray(results[c]["outT"]).T.astype(np.float32)
    return out


def run_spmd_prefill(nc, in_maps, out_prefill, n_cores):
    """Like bass2jax.run_bass_via_pjrt but the donated output buffers are
    prefilled with `out_prefill[name]` per core (the kernel accumulates onto
    outT, which must start as h16T)."""
    import jax
    from jax.sharding import Mesh, PartitionSpec
    from jax.experimental.shard_map import shard_map as shard_map_fn
    import concourse.bass2jax as b2j
    import concourse.mybir as mybir

    b2j.install_neuronx_cc_hook()
    partition_name = nc.partition_id_tensor.name if nc.partition_id_tensor else None
    in_names, out_names, out_avals = [], [], []
    for alloc in nc.m.functions[0].allocations:
        if not isinstance(alloc, mybir.MemoryLocationSet):
            continue
        name = alloc.memorylocations[0].name
        if alloc.kind == "ExternalInput":
            if name != partition_name:
                in_names.append(name)
        elif alloc.kind == "ExternalOutput":
            out_names.append(name)
            out_avals.append(jax.core.ShapedArray(
                tuple(alloc.tensor_shape), mybir.dt.np(alloc.dtype)))
    n_params = len(in_names)
    all_in_names = in_names + out_names
    if partition_name is not None:
        all_in_names = all_in_names + [partition_name]
    donate = tuple(range(n_params, n_params + len(out_names)))

    def _body(*args):
        operands = list(args)
        if partition_name is not None:
            operands.append(b2j.partition_id_tensor())
        outs = b2j._bass_exec_p.bind(
            *operands,
            out_avals=tuple(out_avals), in_names=tuple(all_in_names),
            out_names=tuple(out_names), lowering_input_output_aliases=(),
            sim_require_finite=True, sim_require_nnan=True, nc=nc)
        return tuple(outs)

    devices = jax.devices()[:n_cores]
    mesh = Mesh(np.asarray(devices), ("core",))
    fn = jax.jit(
        shard_map_fn(_body, mesh=mesh,
                     in_specs=(PartitionSpec("core"),) * (n_params + len(out_names)),
                     out_specs=(PartitionSpec("core"),) * len(out_names),
                     check_rep=False),
        donate_argnums=donate, keep_unused=True)
    concat_in = [
        np.concatenate([np.asarray(in_maps[c][nm]) for c in range(n_cores)], axis=0)
        for nm in in_names]
    concat_fill = [
        np.concatenate([np.asarray(out_prefill[c][nm]) for c in range(n_cores)], axis=0)
        for nm in out_names]
    out_arrs = fn(*concat_in, *concat_fill)
    return [
        {nm: np.asarray(out_arrs[i]).reshape(n_cores, *out_avals[i].shape)[c]
         for i, nm in enumerate(out_names)}
        for c in range(n_cores)
    ]


# revision 12
# speedup vs baseline: 1.7605x; 1.0171x over previous
"""AUGRU cell kernel for trn2, 8-core data-parallel, fp8-DoubleRow compute.

Layout: transposed ("feature-major") - features on partitions, batch rows on
the free dim; per-feature gate biases become per-partition ACT biases.

All six gemm-sides (x@Wu, h@Uu, x@Wr, h@Ur, h@Uh, x@Wh) run as fp8-e4m3
DoubleRow matmuls (K=256 per instruction at 0.5 cycles/row = 4x bf16 rate)
with full Dekker compensation per side, using a global x16 weight prescale:

    S*(v @ W) ~= v8 @ Q8(S*W) + v8 @ Q8(S*W - Q8(S*W)) + vr8 @ Q8(W)

where v8 = Q8(v) and vr8 = Q8(S*(v - v8)) carries the S factor. The S scale
keeps the weight residual inside e4m3's normal range and is undone for free
by the activations' input scale (sigmoid/tanh compute f(psum/S + bias)).
The rx side drops its weight residual (17 DR terms per 512 rows); measured
end-to-end error vs the fp32 reference: 9.3e-3 (vs the 2e-2 budget).

Per (1024-row chunk, m-half) iteration, PE order rx,rh,e,ux,uh,g (36 DR
matmuls into 4 psum tiles of 2 banks each = all 8 banks):
  u  = sigmoid(ps_u/S + bu)                  [ScalarE, 2-bank psum read]
  r  = sigmoid(ps_r/S + br)                  [ScalarE]
  s  = a_bc * u                              [DVE stt, fp16 4x]
  t  = r * ps_e                              [DVE stt]
  hp = t + ps_g                              [DVE stt, fp32 out]
  hh = tanh(hp/S + bh)                       [ScalarE]
  d  = hh - h16 ; p = s * d ; out = h16 + p  [DVE stt, fp16 4x]
All pointwise ops are scalar_tensor_tensor (supports the 4x_2p DVE perf
mode; plain tensor_tensor only supports 2x_1p). Intermediates are fp16
(~6x lower rounding error than bf16); output is stored fp16 and upcast on
the host. a is broadcast across partitions by a stride-0 DMA.
"""
import numpy as np
from contextlib import ExitStack

import bass_rust
import ml_dtypes
import concourse.bass as bass
import concourse.mybir as mybir
import concourse.tile as tile
from concourse.bass_utils import run_bass_kernel_spmd

F32 = mybir.dt.float32
F16 = mybir.dt.float16
F8 = mybir.dt.float8e4
E4 = ml_dtypes.float8_e4m3
DR = mybir.MatmulPerfMode.DoubleRow
SIG = mybir.ActivationFunctionType.Sigmoid
TANH = mybir.ActivationFunctionType.Tanh

B, D = 65536, 256
NCORES = 8
BL = B // NCORES          # rows per core
P = 128
CN = 1024                 # rows per chunk
NCHUNK = BL // CN         # 8
S = 16.0                  # global weight prescale

# gemm-side order on the PE: r-gate first so its sigmoid (feeding the long
# t->hp->tanh chain) starts as early as possible.
# (side index, moving per term: 0,1 -> v8, 2 -> vr8)
SIDES = ["rx", "rh", "e", "ux", "uh", "g"]
X_SIDES = {"rx", "ux", "g"}
PS_OF = {"rx": "r", "rh": "r", "ux": "u", "uh": "u", "e": "e", "g": "g"}
NT = 3                    # terms per side
# per-side compensation terms: 0=main Q8(S*W), 1=weight-residual, 2=act-residual.
# rx drops its weight residual and rh its act residual (measured composite
# rel err 1.31e-2 vs the 2e-2 budget).
TERMS = {"rx": [0, 2], "rh": [0, 1], "e": [0, 1, 2],
         "ux": [0, 1, 2], "uh": [0, 1, 2], "g": [0, 1, 2]}


def split_multi_waits(nc):
    """Walrus codegen allows at most one semaphore wait per instruction.
    Split extras onto preceding same-engine NoOps."""
    for fn in nc.m.functions:
        for bb in fn.blocks:
            out = []
            for inst in bb.instructions:
                si = inst.sync_info
                if si is not None and len(si.on_wait) > 1:
                    waits = list(si.on_wait)
                    for j, w in enumerate(waits[:-1]):
                        nop = bass_rust.InstNoOp(name=f"{inst.name}-sw{j}")
                        nop.engine = inst.engine
                        nop.sync_info = mybir.SyncInfo(on_wait=[w], on_update=[])
                        out.append(nop)
                    inst.sync_info = mybir.SyncInfo(
                        on_wait=[waits[-1]], on_update=list(si.on_update))
                out.append(inst)
            bb.instructions = out


def build():
    nc = bass.Bass()
    c_d = nc.declare_dram_parameter("consts8", [P, 6 * NT * 2 * 256], F8, isOutput=False)
    b_d = nc.declare_dram_parameter("bias", [P, 6], F32, isOutput=False)
    x8_d = nc.declare_dram_parameter("x8T", [D, BL], F8, isOutput=False)
    xr_d = nc.declare_dram_parameter("xrT", [D, BL], F8, isOutput=False)
    h8_d = nc.declare_dram_parameter("h8T", [D, BL], F8, isOutput=False)
    hr_d = nc.declare_dram_parameter("hrT", [D, BL], F8, isOutput=False)
    h16_d = nc.declare_dram_parameter("h16T", [D, BL], F16, isOutput=False)
    a_d = nc.declare_dram_parameter("aT", [1, BL], F16, isOutput=False)
    o_d = nc.declare_dram_parameter("outT", [D, BL], F16, isOutput=True)

    c_ap = c_d.ap().rearrange("p (h s t k m) -> p h s t k m", h=2, s=6, t=NT, k=2)
    aps = {"x8": x8_d.ap().rearrange("(c p) n -> p c n", p=P),
           "xr": xr_d.ap().rearrange("(c p) n -> p c n", p=P),
           "h8": h8_d.ap().rearrange("(c p) n -> p c n", p=P),
           "hr": hr_d.ap().rearrange("(c p) n -> p c n", p=P),
           "h16": h16_d.ap().rearrange("(c p) n -> p c n", p=P)}
    o_ap = o_d.ap().rearrange("(c p) n -> p c n", p=P)

    # chunk schedule: small first/last chunks for fast pipeline fill/drain
    chunks = []
    n0 = 0
    for cn in [512] + [1024] * 7 + [512]:
        chunks.append((n0, cn))
        n0 += cn
    assert n0 == BL

    dt_of = {"x8": F8, "xr": F8, "h8": F8, "hr": F8, "h16": F16}

    with tile.TileContext(nc) as tc, ExitStack() as ctx:
        const = ctx.enter_context(tc.tile_pool(name="const", bufs=1))
        io = ctx.enter_context(tc.tile_pool(name="io", bufs=1))
        sm = ctx.enter_context(tc.tile_pool(name="sm", bufs=1))
        psum = ctx.enter_context(tc.tile_pool(name="psum", bufs=1, space="PSUM"))

        c_sb = const.tile([P, 2, 6, NT, 2, 128], F8)
        b_sb = const.tile([P, 6], F32)

        def load_one(nm, ci, n0, cn):
            t_ = io.tile([P, 2, CN], dt_of[nm], tag=nm, bufs=4, name=nm)
            _lbl(nc.sync.dma_start(out=t_[:, :, 0:cn], in_=aps[nm][:, :, n0:n0 + cn]),
                 f"dma.{nm}.c{ci}")
            return t_

        def load_abc(ci, n0, cn):
            a_bc = io.tile([P, CN], F16, tag="a_bc", bufs=4, name="a_bc")
            _lbl(nc.sync.dma_start(
                out=a_bc[:, 0:cn],
                in_=a_d.ap()[0:1, n0:n0 + cn].to_broadcast((P, cn))), f"dma.a.c{ci}")
            return a_bc

        # staged startup: weights interleaved with first-chunk data in PE need order
        n0_0, cn_0 = chunks[0]
        nc.sync.dma_start(out=c_sb[:, 0:1, 0:3], in_=c_ap[:, 0:1, 0:3])  # m0 rx,rh,e w
        t0 = {"x8": load_one("x8", 0, n0_0, cn_0)}
        t0["h8"] = load_one("h8", 0, n0_0, cn_0)
        t0["xr"] = load_one("xr", 0, n0_0, cn_0)
        t0["hr"] = load_one("hr", 0, n0_0, cn_0)
        nc.sync.dma_start(out=c_sb[:, 0:1, 3:6], in_=c_ap[:, 0:1, 3:6])  # m0 ux,uh,g w
        nc.sync.dma_start(out=b_sb, in_=b_d.ap())
        nc.sync.dma_start(out=c_sb[:, 1:2], in_=c_ap[:, 1:2])        # m1 weights
        t0["h16"] = load_one("h16", 0, n0_0, cn_0)
        t0["a_bc"] = load_abc(0, n0_0, cn_0)

        for ci, (n0, cn) in enumerate(chunks):
            nj = max(1, cn // 512)
            if ci == 0:
                tl = t0
            else:
                tl = {nm: load_one(nm, ci, n0, cn) for nm in
                      ("x8", "h8", "xr", "hr", "h16")}
                tl["a_bc"] = load_abc(ci, n0, cn)
            mv_of = {0: {True: tl["x8"], False: tl["h8"]},
                     2: {True: tl["xr"], False: tl["hr"]}}
            mv_of[1] = mv_of[0]
            s16 = sm.tile([P, 2, CN], F16, tag="s16", bufs=2, name="s16")
            hh = sm.tile([P, 2, CN], F16, tag="hh", bufs=2, name="hh")
            for m in range(2):
                ps = {k: psum.tile([P, 2, 512], F32, tag=f"ps_{k}", bufs=1,
                                   name=f"ps_{k}")
                      for k in ("r", "u", "e", "g")}
                for si, side in enumerate(SIDES):
                    isx = side in X_SIDES
                    pst = ps[PS_OF[side]]
                    terms = TERMS[side]
                    two_sided = side[0] in ("r", "u")
                    first_side = side in ("rx", "ux") or not two_sided
                    last_side = side in ("rh", "uh") or not two_sided
                    for j in range(nj):
                        jw = min(512, cn - j * 512)
                        js = slice(j * 512, j * 512 + jw)
                        for t_ in terms:
                            mv = mv_of[t_][isx]
                            _lbl(nc.tensor.matmul(
                                pst[:, j, 0:jw], c_sb[:, m, si, t_, :, :],
                                mv[:, :, js],
                                start=(first_side and t_ == terms[0]),
                                stop=(last_side and t_ == terms[-1]),
                                perf_mode=DR), f"mm.{side}.{t_}.j{j}.c{ci}m{m}")

                ps_flat = {k: v.rearrange("p a b -> p (a b)")[:, 0:cn]
                           for k, v in ps.items()}
                r16 = sm.tile([P, CN], F16, tag="r16", bufs=2, name="r16")
                _lbl(nc.scalar.activation(r16[:, 0:cn], ps_flat["r"], SIG,
                                          bias=b_sb[:, 2 + m:3 + m], scale=1.0 / S),
                     f"act.r.c{ci}m{m}")
                u16 = sm.tile([P, CN], F16, tag="u16", bufs=2, name="u16")
                _lbl(nc.scalar.activation(u16[:, 0:cn], ps_flat["u"], SIG,
                                          bias=b_sb[:, 0 + m:1 + m], scale=1.0 / S),
                     f"act.u.c{ci}m{m}")
                t16 = sm.tile([P, CN], F16, tag="t16", bufs=2, name="t16")
                _lbl(nc.vector.tensor_mul(out=t16[:, 0:cn], in0=r16[:, 0:cn],
                                          in1=ps_flat["e"]), f"dve.t.c{ci}m{m}")
                _lbl(nc.gpsimd.tensor_mul(out=s16[:, m, 0:cn], in0=tl["a_bc"][:, 0:cn],
                                          in1=u16[:, 0:cn]), f"pool.s.c{ci}m{m}")
                hp = sm.tile([P, CN], F32, tag="hp", bufs=2, name="hp")
                _lbl(nc.vector.tensor_add(out=hp[:, 0:cn], in0=t16[:, 0:cn],
                                          in1=ps_flat["g"]), f"dve.hp.c{ci}m{m}")
                _lbl(nc.scalar.activation(hh[:, m, 0:cn], hp[:, 0:cn], TANH,
                                          bias=b_sb[:, 4 + m:5 + m], scale=1.0 / S),
                     f"act.tanh.c{ci}m{m}")

            # blend per m-half right after each tanh (keeps the drain tail short)
            last = ci == len(chunks) - 1
            for m in range(2):
                d16 = sm.tile([P, CN], F16, tag="d16", bufs=2, name="d16")
                _lbl(nc.vector.tensor_sub(out=d16[:, 0:cn], in0=hh[:, m, 0:cn],
                                          in1=tl["h16"][:, m, 0:cn]), f"dve.d.c{ci}m{m}")
                p16 = sm.tile([P, CN], F16, tag="p16", bufs=2, name="p16")
                _lbl(nc.vector.tensor_mul(out=p16[:, 0:cn], in0=s16[:, m, 0:cn],
                                          in1=d16[:, 0:cn]), f"dve.p.c{ci}m{m}")
                if not last:
                    # out = h16 + p via fp16 accumulating DMA onto prefilled outT
                    _lbl(nc.gpsimd.dma_start(
                        out=o_ap[:, m, n0:n0 + cn],
                        in_=p16[:, 0:cn],
                        accum_op=mybir.AluOpType.add), f"dma.out.c{ci}m{m}")
                else:
                    # drain tail: DVE out-add + plain store (no SWDGE chain)
                    o16 = sm.tile([P, CN], F16, tag="o16l", bufs=2, name="o16l")
                    _lbl(nc.vector.tensor_add(out=o16[:, 0:cn],
                                              in0=tl["h16"][:, m, 0:cn],
                                              in1=p16[:, 0:cn]), f"dve.o.c{ci}m{m}")
                    _lbl(nc.sync.dma_start(out=o_ap[:, m, n0:n0 + cn],
                                           in_=o16[:, 0:cn]), f"dma.out.c{ci}m{m}")

    split_multi_waits(nc)
    return nc


def q8(v):
    return np.asarray(v, np.float32).astype(E4)


def pack_consts(Wu, Uu, bu, Wr, Ur, br, Wh, Uh, bh):
    """consts8 [P, 2, 6, 3, 2, 128] fp8 (m-half major) + bias [P, 6] fp32."""
    w_of = {"rx": Wr, "rh": Ur, "e": Uh, "ux": Wu, "uh": Uu, "g": Wh}
    out = np.zeros((P, 2, 6, NT, 2, 128), E4)

    def ktile(arr):  # [256, 256] -> [128, 2, 256]
        return np.asarray(arr).reshape(2, P, 256).transpose(1, 0, 2)

    for si, side in enumerate(SIDES):
        W = np.asarray(w_of[side], np.float32)
        main = q8(S * W)
        wres = q8(S * W - main.astype(np.float32))
        acres = q8(W)
        for t_, arr in enumerate((main, wres, acres)):
            kt = ktile(arr)           # [128, 2, 256]
            for mh in range(2):
                out[:, mh, si, t_] = kt[:, :, mh * 128:(mh + 1) * 128]

    bias = np.zeros((P, 6), np.float32)
    for gi, bv in enumerate((bu, br, bh)):
        bias[:, 2 * gi:2 * gi + 2] = np.asarray(bv, np.float32).reshape(2, P).T
    return np.ascontiguousarray(out.reshape(P, -1)), np.ascontiguousarray(bias)


_CACHE = {}
LABELS = {}


def _lbl(inst, label):
    try:
        LABELS[inst.name] = label
    except Exception:
        pass
    return inst


def _get_nc():
    if "nc" not in _CACHE:
        _CACHE["nc"] = build()
    return _CACHE["nc"]


def kernel(x, h_1, a, Wu, Uu, bu, Wr, Ur, br, Wh, Uh, bh):
    nc = _get_nc()
    consts8, bias = pack_consts(Wu, Uu, bu, Wr, Ur, br, Wh, Uh, bh)
    x = np.asarray(x, np.float32)
    h = np.asarray(h_1, np.float32)
    a = np.asarray(a, np.float32)

    x8 = x.astype(E4)
    xr = (S * (x - x8.astype(np.float32))).astype(E4)
    h8 = h.astype(E4)
    hr = (S * (h - h8.astype(np.float32))).astype(E4)
    h16 = h.astype(np.float16)

    in_maps = []
    for c in range(NCORES):
        sl = slice(c * BL, (c + 1) * BL)
        in_maps.append({
            "consts8": consts8,
            "bias": bias,
            "x8T": np.ascontiguousarray(x8[sl].T),
            "xrT": np.ascontiguousarray(xr[sl].T),
            "h8T": np.ascontiguousarray(h8[sl].T),
            "hrT": np.ascontiguousarray(hr[sl].T),
            "h16T": np.ascontiguousarray(h16[sl].T),
            "aT": np.ascontiguousarray(a[sl].T).astype(np.float16),
        })
    prefills = [{"outT": im["h16T"]} for im in in_maps]
    results = run_spmd_prefill(nc, in_maps, prefills, NCORES)
    out = np.empty((B, D), np.float32)
    for c in range(NCORES):
        out[c * BL:(c + 1) * BL] = np.asarray(results[c]["outT"]).T.astype(np.float32)
    return out


def run_spmd_prefill(nc, in_maps, out_prefill, n_cores):
    """Like bass2jax.run_bass_via_pjrt but the donated output buffers are
    prefilled with `out_prefill[name]` per core (the kernel accumulates onto
    outT, which must start as h16T)."""
    import jax
    from jax.sharding import Mesh, PartitionSpec
    from jax.experimental.shard_map import shard_map as shard_map_fn
    import concourse.bass2jax as b2j
    import concourse.mybir as mybir

    b2j.install_neuronx_cc_hook()
    partition_name = nc.partition_id_tensor.name if nc.partition_id_tensor else None
    in_names, out_names, out_avals = [], [], []
    for alloc in nc.m.functions[0].allocations:
        if not isinstance(alloc, mybir.MemoryLocationSet):
            continue
        name = alloc.memorylocations[0].name
        if alloc.kind == "ExternalInput":
            if name != partition_name:
                in_names.append(name)
        elif alloc.kind == "ExternalOutput":
            out_names.append(name)
            out_avals.append(jax.core.ShapedArray(
                tuple(alloc.tensor_shape), mybir.dt.np(alloc.dtype)))
    n_params = len(in_names)
    all_in_names = in_names + out_names
    if partition_name is not None:
        all_in_names = all_in_names + [partition_name]
    donate = tuple(range(n_params, n_params + len(out_names)))

    def _body(*args):
        operands = list(args)
        if partition_name is not None:
            operands.append(b2j.partition_id_tensor())
        outs = b2j._bass_exec_p.bind(
            *operands,
            out_avals=tuple(out_avals), in_names=tuple(all_in_names),
            out_names=tuple(out_names), lowering_input_output_aliases=(),
            sim_require_finite=True, sim_require_nnan=True, nc=nc)
        return tuple(outs)

    devices = jax.devices()[:n_cores]
    mesh = Mesh(np.asarray(devices), ("core",))
    fn = jax.jit(
        shard_map_fn(_body, mesh=mesh,
                     in_specs=(PartitionSpec("core"),) * (n_params + len(out_names)),
                     out_specs=(PartitionSpec("core"),) * len(out_names),
                     check_rep=False),
        donate_argnums=donate, keep_unused=True)
    concat_in = [
        np.concatenate([np.asarray(in_maps[c][nm]) for c in range(n_cores)], axis=0)
        for nm in in_names]
    concat_fill = [
        np.concatenate([np.asarray(out_prefill[c][nm]) for c in range(n_cores)], axis=0)
        for nm in out_names]
    out_arrs = fn(*concat_in, *concat_fill)
    return [
        {nm: np.asarray(out_arrs[i]).reshape(n_cores, *out_avals[i].shape)[c]
         for i, nm in enumerate(out_names)}
        for c in range(n_cores)
    ]
